# revision 46
# baseline (speedup 1.0000x reference)
"""Trainium2 Bass kernel for nn_DecoderBlock (self-attn + cross-attn + FFN, post-LN).

Sharding (8 cores = 2 batch groups x 4 cores):
 - Attention is head-parallel within each group (4 heads/core, full S), which
   keeps the causal-mask instruction stream rank-uniform (SPMD).
 - Everything else (out-projections, LayerNorms, FFN) is sequence-parallel:
   each core owns a 512-token strip and uses the FULL Wo/W1/W2 weights, so
   there are no partial sums and no AllReduces.
 - All resharding transitions (attention-out head->strip, cross-Q
   strip->head) are 8-core AllToAlls of ~2MB, combined on the receiver with
   a per-core 0/1 mask (SPMD keeps all addressing rank-uniform; rank
   variation lives entirely in host-supplied data).
 - All matmul operands are bf16 (fp32 PSUM accumulation); the residual/LN
   stream is kept fp32.
"""

import os
import sys

sys.path.insert(0, "/opt/trn_rl_repo")

from contextlib import ExitStack

import ml_dtypes
import numpy as np

import concourse.bacc as bacc
import concourse.tile as tile
from concourse import mybir
from concourse.bass_utils import run_bass_kernel_spmd

F32R = mybir.dt.float32r
F32 = mybir.dt.float32
BF16 = mybir.dt.bfloat16
AF = mybir.ActivationFunctionType
ALU = mybir.AluOpType

B = 2
D = 1024
H = 16
HD = 64
FF = 4 * D
NCORES = 8
G = 4                  # cores per batch group
HL = H // G            # 4 local heads
DC = HL * HD           # 256 local q/k/v features
DCA = HL * (HD + 1)    # 260: V augmented with a ones column per head
SW = 512               # tokens per core strip
CW = 512               # free-dim chunk width
DT = D // 128          # 8 feature partition-tiles
FT = FF // 128         # 32 ffn partition-tiles
GROUPS4 = [[0, 1, 2, 3], [4, 5, 6, 7]]
GROUP8 = [[0, 1, 2, 3, 4, 5, 6, 7]]

_nc_cache = {}


def _build(S, M, causal):
    nc = bacc.Bacc(None, target_bir_lowering=False, num_devices=NCORES)
    NCH = S // CW          # 4 query chunks (full S)
    MT = M // 128          # 16 key tiles (cross)
    ST = S // 128          # 16 key tiles (self)

    dp = nc.declare_dram_parameter
    xT = dp("xT", [D, S], BF16, isOutput=False)
    xs = dp("xs", [D, SW], F32R, isOutput=False)
    memT = dp("memT", [D, M], BF16, isOutput=False)
    wq = dp("wq", [D, DC], BF16, isOutput=False)
    wk = dp("wk", [D, DC], BF16, isOutput=False)
    wv = dp("wv", [D, DCA], BF16, isOutput=False)
    bq = dp("bq", [DC, 1], F32, isOutput=False)
    bk = dp("bk", [DC, 1], F32, isOutput=False)
    bva = dp("bva", [1, DCA], F32, isOutput=False)
    wo = dp("wo", [D, D], BF16, isOutput=False)
    bo = dp("bo", [D, 1], F32, isOutput=False)
    wqx = dp("wqx", [D, D], BF16, isOutput=False)
    wkx = dp("wkx", [D, DC], BF16, isOutput=False)
    wvx = dp("wvx", [D, DCA], BF16, isOutput=False)
    bqx = dp("bqx", [D, 1], F32, isOutput=False)
    bkx = dp("bkx", [DC, 1], F32, isOutput=False)
    bvxa = dp("bvxa", [1, DCA], F32, isOutput=False)
    wox = dp("wox", [D, D], BF16, isOutput=False)
    box = dp("box", [D, 1], F32, isOutput=False)
    w1 = dp("w1", [D, FF], BF16, isOutput=False)
    b1 = dp("b1", [FF, 1], F32, isOutput=False)
    w2 = dp("w2", [FF, D], BF16, isOutput=False)
    b2 = dp("b2", [D, 1], F32, isOutput=False)
    lng = dp("lng", [3 * D, 1], F32, isOutput=False)
    lnb = dp("lnb", [3 * D, 1], F32, isOutput=False)
    ones = dp("ones", [128, 128], F32R, isOutput=False)
    gmask = dp("gmask", [128, 2], F32, isOutput=False)
    gmask4 = dp("gmask4", [128, 4], F32, isOutput=False)
    outT = dp("outT", [D, SW], F32R, isOutput=True)

    with tile.TileContext(nc) as tc, ExitStack() as st:
        ep = st.enter_context
        constp = ep(tc.tile_pool(name="const", bufs=1))
        dramp = ep(tc.tile_pool(name="dram", bufs=1, space="DRAM"))

        DMA_ENG = [nc.sync, nc.gpsimd, nc.scalar]
        # gpsimd's queue sits behind the collectives-prelude barrier; route
        # startup loads to the HWDGE engines only.
        LOAD_ENG = [nc.sync, nc.scalar]

        # ---- DRAM bounce buffers for collectives ----
        t1in = dramp.tile([2 * D, SW], BF16, name="t1in")
        t1out = dramp.tile([2 * D, SW], BF16, name="t1out")
        q2in = dramp.tile([2 * D, SW], BF16, name="q2in")
        q2out = dramp.tile([2 * D, SW], BF16, name="q2out")
        t3in = dramp.tile([2 * D, SW], BF16, name="t3in")
        t3out = dramp.tile([2 * D, SW], BF16, name="t3out")
        t3in2 = dramp.tile([2 * D, SW], BF16, name="t3in2")
        t3out2 = dramp.tile([2 * D, SW], BF16, name="t3out2")

        # ---- constants ----
        ones_t = constp.tile([128, 128], F32R, name="ones_t")
        nc.gpsimd.dma_start(out=ones_t[:], in_=ones[:, :])
        eps_t = constp.tile([128, 1], F32, name="eps_t")
        nc.vector.memset(eps_t[:], 1e-5)
        gmask_t = constp.tile([128, 2], F32, name="gmask_t")
        nc.gpsimd.dma_start(out=gmask_t[:], in_=gmask[:, :])
        gmask4_t = constp.tile([128, 4], F32, name="gmask4_t")
        nc.gpsimd.dma_start(out=gmask4_t[:], in_=gmask4[:, :])

        def bias_tiles(src, n, prefix):
            ts = []
            for i in range(n):
                t = constp.tile([128, 1], F32, name=f"{prefix}{i}")
                nc.gpsimd.dma_start(out=t[:], in_=src[i * 128:(i + 1) * 128, :])
                ts.append(t)
            return ts

        bq_t = bias_tiles(bq, 2, "bq")
        bk_t = bias_tiles(bk, 2, "bk")
        bqx_t = bias_tiles(bqx, DT, "bqx")
        bkx_t = bias_tiles(bkx, 2, "bkx")
        bo_t = bias_tiles(bo, DT, "bo")
        box_t = bias_tiles(box, DT, "box")
        b1_t = bias_tiles(b1, FT, "b1")
        b2_t = bias_tiles(b2, DT, "b2")
        lng_t = bias_tiles(lng, 3 * DT, "lng")
        lnb_t = bias_tiles(lnb, 3 * DT, "lnb")
        bva_t = constp.tile([128, DCA], F32, name="bva_t")
        nc.gpsimd.dma_start(out=bva_t[:], in_=bva[:, :].to_broadcast([128, DCA]))
        bvxa_t = constp.tile([128, DCA], F32, name="bvxa_t")
        nc.gpsimd.dma_start(out=bvxa_t[:],
                            in_=bvxa[:, :].to_broadcast([128, DCA]))

        # ================= helpers =================
        _wload_rr = [0]

        def load_w_small(pool, src, cols, tag):
            """Whole [DT*128, cols] weight in ONE DMA -> [128, DT*cols] tile;
            returns per-d [128, cols] slice APs."""
            t = pool.tile([128, DT * cols], BF16, name=tag, tag=tag, bufs=1)
            eng = LOAD_ENG[_wload_rr[0] % len(LOAD_ENG)]
            _wload_rr[0] += 1
            eng.dma_start(
                out=t[:].rearrange("p (d j) -> p d j", j=cols),
                in_=src[:, :].rearrange("(d p) j -> p d j", p=128))
            return [t[:, d * cols:(d + 1) * cols] for d in range(DT)]

        def load_w_rows(pool, src, cols, tag, nrows):
            """[128, cols] row-tiles of a big weight (rows = contraction)."""
            ts = []
            for k in range(nrows):
                t = pool.tile([128, cols], BF16, name=tag, tag=tag, bufs=nrows)
                DMA_ENG[k % len(DMA_ENG)].dma_start(
                    out=t[:], in_=src[k * 128:(k + 1) * 128, :])
                ts.append(t)
            return ts

        def project_qk(qt_pair, w_tiles, b_tiles, src_tiles, psum, tag,
                       interleave=None):
            """q/k projection: 2x [128, S] packed tiles (2 heads each)."""
            it = iter(interleave) if interleave is not None else None
            for sc in range(NCH):
                sl = slice(sc * CW, (sc + 1) * CW)
                for t in range(2):
                    ps = psum.tile([128, CW], F32, name=f"{tag}ps", tag="qkps",
                                   bufs=2)
                    for d in range(DT):
                        nc.tensor.matmul(
                            out=ps[:],
                            lhsT=w_tiles[d][:, t * 128:(t + 1) * 128],
                            rhs=src_tiles[d][:, sl],
                            start=(d == 0), stop=(d == DT - 1),
                        )
                    nc.scalar.activation(out=qt_pair[t][:, sl], in_=ps[:],
                                         func=AF.Identity, bias=b_tiles[t][:],
                                         scale=1.0)
                if it is not None:
                    next(it, None)

        def project_v(vpool, w_tiles, bias_bc, src_tiles, psum, n_tok, tag,
                      interleave=None):
            """v projection: n_tok/128 tiles of [128 tok, DCA]."""
            it = iter(interleave) if interleave is not None else None
            vs = []
            for s_t in range(n_tok // 128):
                ps = psum.tile([128, DCA], F32, name=f"{tag}ps", tag="vps",
                               bufs=2)
                for d in range(DT):
                    nc.tensor.matmul(
                        out=ps[:],
                        lhsT=src_tiles[d][:, s_t * 128:(s_t + 1) * 128],
                        rhs=w_tiles[d][:],
                        start=(d == 0), stop=(d == DT - 1))
                vt = vpool.tile([128, DCA], BF16, name=f"{tag}v", tag="vs",
                                bufs=2 * ST)
                nc.vector.tensor_tensor(out=vt[:], in0=ps[:], in1=bias_bc[:],
                                        op=ALU.add)
                vs.append(vt)
                if it is not None and s_t % 2 == 1:
                    next(it, None)
            return vs

        def attention(apool, ppool, dpool, q_tiles, k_tiles, v_tiles, n_keys,
                      use_mask, psc, ppv, tag, chunk_cb=None):
            """Full-S head-sharded attention; returns 2 packed [128, S]
            bf16 tiles (2 heads each). The two heads of a pair run their
            score matmuls on disjoint PE row-groups (concurrent)."""
            a_packed = [apool.tile([128, S], BF16, name=f"{tag}{t}", tag="attn",
                                   bufs=2) for t in range(2)]
            kt_total = n_keys // 128
            kpc = CW // 128
            for qc in range(NCH):
                sl = slice(qc * CW, (qc + 1) * CW)
                for ti in range(HL // 2):
                    kts = range(min(kt_total, kpc * (qc + 1)) if use_mask
                                else kt_total)
                    n_kt = len(kts)
                    pv_ps = [ppv.tile([65, CW], F32, name=f"{tag}pv{par}",
                                      tag="pvps", bufs=4) for par in range(2)]

                    def emit_scores(kt):
                        s_ps = [psc.tile([128, CW], F32, name=f"{tag}s{par}",
                                         tag="scps", bufs=4)
                                for par in range(2)]
                        for par in range(2):
                            nc.tensor.matmul(
                                out=s_ps[par][:],
                                lhsT=k_tiles[ti][par * 64:(par + 1) * 64,
                                                 kt * 128:(kt + 1) * 128],
                                rhs=q_tiles[ti][par * 64:(par + 1) * 64, sl],
                                start=True, stop=True,
                                tile_position=(64 * par, 0),
                            )
                        return s_ps

                    def emit_pv(s_ps, kt, i):
                        for par in range(2):
                            h = 2 * ti + par
                            p_t = ppool.tile([128, CW], BF16, name=f"{tag}p",
                                             tag="p", bufs=6)
                            nc.scalar.activation(out=p_t[:], in_=s_ps[par][:],
                                                 func=AF.Exp)
                            if use_mask and kt >= kpc * qc:
                                p_m = ppool.tile([128, CW], BF16,
                                                 name=f"{tag}pm", tag="p",
                                                 bufs=6)
                                nc.gpsimd.affine_select(
                                    out=p_m[:], in_=p_t[:], pattern=[[1, CW]],
                                    compare_op=ALU.is_ge, fill=0.0,
                                    base=qc * CW - kt * 128,
                                    channel_multiplier=-1)
                                p_use = p_m
                            else:
                                p_use = p_t
                            nc.tensor.matmul(
                                out=pv_ps[par][:],
                                lhsT=v_tiles[kt][:, h * 65:(h + 1) * 65],
                                rhs=p_use[:],
                                start=(i == 0), stop=(i == n_kt - 1),
                            )

                    # one-step lookahead: scores of kt+1 are emitted before
                    # the exp-gated PV of kt so the PE never queues behind
                    # a semaphore wait on the scalar engine
                    prev = None
                    for i, kt in enumerate(kts):
                        s_ps = emit_scores(kt)
                        if prev is not None:
                            emit_pv(*prev)
                        prev = (s_ps, kt, i)
                    emit_pv(*prev)
                    for par in range(2):
                        # softmax denominator: reciprocal of row 64, broadcast
                        drc = dpool.tile([65, CW], F32, name=f"{tag}drc",
                                         tag="drc", bufs=2)
                        nc.vector.reciprocal(out=drc[64:65, :],
                                             in_=pv_ps[par][64:65, :])
                        dn0 = dpool.tile([1, CW], F32, name=f"{tag}dn0",
                                         tag="dn0", bufs=2)
                        nc.sync.dma_start(out=dn0[0:1, :], in_=drc[64:65, :])
                        db = dpool.tile([64, CW], F32, name=f"{tag}db",
                                        tag="db", bufs=2)
                        nc.gpsimd.partition_broadcast(db[:], dn0[0:1, :])
                        nc.vector.tensor_tensor(
                            out=a_packed[ti][par * 64:(par + 1) * 64, sl],
                            in0=pv_ps[par][0:64, :], in1=db[:], op=ALU.mult)
            return a_packed

        def a2a_send(a_packed, zin, zout):
            """Spill the packed attention output and kick off the AllToAll."""
            for j in range(2 * G):
                for ti in range(2):
                    DMA_ENG[(2 * j + ti) % len(DMA_ENG)].dma_start(
                        out=zin[j * 256 + ti * 128: j * 256 + (ti + 1) * 128, :],
                        in_=a_packed[ti][:, (j % G) * SW:(j % G + 1) * SW])
            nc.gpsimd.collective_compute(
                "AllToAll", ALU.bypass, replica_groups=GROUP8,
                ins=[zin.opt()], outs=[zout.opt()])

        def a2a_send_half(a_packed, zin, zout, base_strip):
            """Spill strips [base_strip, base_strip+2) and AllToAll them."""
            for j in range(2 * G):
                for ti in range(2):
                    s0 = (base_strip + (j % 2)) * SW
                    DMA_ENG[(2 * j + ti) % len(DMA_ENG)].dma_start(
                        out=zin[j * 256 + ti * 128: j * 256 + (ti + 1) * 128, :],
                        in_=a_packed[ti][:, s0:s0 + SW])
            nc.gpsimd.collective_compute(
                "AllToAll", ALU.bypass, replica_groups=GROUP8,
                ins=[zin.opt()], outs=[zout.opt()])

        def a2a_recv4(zout_a, zout_b, tmp_pool, a_pool, tag):
            """Strip assembly from the two half AllToAlls (4-way mask)."""
            a_str = []
            for ft in range(DT):
                srcs = [(zout_a, 0), (zout_a, D), (zout_b, 0), (zout_b, D)]
                loads = []
                for idx, (z, off) in enumerate(srcs):
                    t = tmp_pool.tile([128, SW], BF16, name=f"{tag}r{idx}",
                                      tag="atmp", bufs=8)
                    DMA_ENG[(ft + idx) % len(DMA_ENG)].dma_start(
                        out=t[:], in_=z[off + ft * 128: off + (ft + 1) * 128, :])
                    loads.append(t)
                a = a_pool.tile([128, SW], BF16, name=f"{tag}a", tag="astr",
                                bufs=DT)
                nc.vector.tensor_scalar_mul(out=a[:], in0=loads[0][:],
                                            scalar1=gmask4_t[:, 0:1])
                for idx in range(1, 4):
                    nc.vector.scalar_tensor_tensor(
                        out=a[:], in0=loads[idx][:],
                        scalar=gmask4_t[:, idx:idx + 1], in1=a[:],
                        op0=ALU.mult, op1=ALU.add)
                a_str.append(a)
            return a_str

        def a2a_recv(zout, tmp_pool, a_pool, tag):
            """Read back my strip: masked add of the two group blocks."""
            a_str = []
            for ft in range(DT):
                top = tmp_pool.tile([128, SW], BF16, name=f"{tag}t", tag="atmp",
                                    bufs=8)
                bot = tmp_pool.tile([128, SW], BF16, name=f"{tag}b", tag="atmp",
                                    bufs=8)
                DMA_ENG[ft % len(DMA_ENG)].dma_start(
                    out=top[:], in_=zout[ft * 128:(ft + 1) * 128, :])
                DMA_ENG[(ft + 1) % len(DMA_ENG)].dma_start(
                    out=bot[:], in_=zout[D + ft * 128:D + (ft + 1) * 128, :])
                a = a_pool.tile([128, SW], BF16, name=f"{tag}a", tag="astr",
                                bufs=DT)
                nc.vector.tensor_scalar_mul(out=a[:], in0=top[:],
                                            scalar1=gmask_t[:, 0:1])
                nc.vector.scalar_tensor_tensor(
                    out=a[:], in0=bot[:], scalar=gmask_t[:, 1:2], in1=a[:],
                    op0=ALU.mult, op1=ALU.add)
                a_str.append(a)
            return a_str

        def out_project_strip(wo_tiles, b_tiles, a_str, z32, psum, tag):
            """z32[d] = wo.T @ a_str + bias + z32  (residual add in place)."""
            for d in range(DT):
                ps = psum.tile([128, CW], F32, name=f"{tag}ps", tag="ops",
                               bufs=2)
                for kt in range(DT):
                    nc.tensor.matmul(
                        out=ps[:], lhsT=wo_tiles[kt][:, d * 128:(d + 1) * 128],
                        rhs=a_str[kt][:],
                        start=(kt == 0), stop=(kt == DT - 1))
                nc.vector.scalar_tensor_tensor(
                    out=z32[d][:], in0=ps[:], scalar=b_tiles[d][:],
                    in1=z32[d][:], op0=ALU.add, op1=ALU.add)

        def layer_norm_strip(z32, z16, ln_idx, psum, sqpool, tmppool, tag):
            """Post-LN on the [D, SW] fp32 strip; writes bf16 copy z16."""
            mps = psum.tile([128, CW], F32, name=f"{tag}m", tag="lnps", bufs=2)
            for d in range(DT):
                nc.tensor.matmul(out=mps[:], lhsT=ones_t[:],
                                 rhs=z32[d][:],
                                 start=(d == 0), stop=(d == DT - 1))
            mu = tmppool.tile([128, CW], F32, name=f"{tag}mu", tag="mu", bufs=1)
            nc.vector.tensor_copy(out=mu[:], in_=mps[:])
            qps = psum.tile([128, CW], F32, name=f"{tag}q", tag="lnps", bufs=2)
            for d in range(DT):
                sq = sqpool.tile([128, CW], F32R, name=f"{tag}sq", tag="sq",
                                 bufs=2)
                nc.scalar.activation(out=sq[:], in_=z32[d][:], func=AF.Square)
                nc.tensor.matmul(out=qps[:], lhsT=ones_t[:], rhs=sq[:],
                                 start=(d == 0), stop=(d == DT - 1))
            var = tmppool.tile([128, CW], F32, name=f"{tag}v", tag="tv", bufs=1)
            nc.vector.tensor_tensor(out=var[:], in0=mu[:], in1=mu[:],
                                    op=ALU.mult)
            nc.vector.tensor_tensor(out=var[:], in0=qps[:], in1=var[:],
                                    op=ALU.subtract)
            std = tmppool.tile([128, CW], F32, name=f"{tag}sd", tag="std",
                               bufs=1)
            nc.scalar.activation(out=std[:], in_=var[:], func=AF.Sqrt,
                                 bias=eps_t[:], scale=1.0)
            rstd = tmppool.tile([128, CW], F32, name=f"{tag}r", tag="rstd",
                                bufs=1)
            nc.vector.reciprocal(out=rstd[:], in_=std[:])
            for d in range(DT):
                xm = tmppool.tile([128, CW], F32, name=f"{tag}x", tag="xm",
                                  bufs=2)
                nc.vector.tensor_tensor(out=xm[:], in0=z32[d][:], in1=mu[:],
                                        op=ALU.subtract)
                nc.vector.tensor_tensor(out=xm[:], in0=xm[:], in1=rstd[:],
                                        op=ALU.mult)
                nc.vector.tensor_scalar(
                    out=z32[d][:], in0=xm[:],
                    scalar1=lng_t[ln_idx * DT + d][:],
                    scalar2=lnb_t[ln_idx * DT + d][:],
                    op0=ALU.mult, op1=ALU.add)
                if z16 is not None:
                    nc.scalar.activation(out=z16[d][:], in_=z32[d][:],
                                         func=AF.Identity)

        # ================= pipeline =================
        # Long-lived pools first (strict LIFO pool order is required).
        zp = ep(tc.tile_pool(name="zp", bufs=DT))
        z16p = ep(tc.tile_pool(name="z16p", bufs=DT))
        wop = ep(tc.tile_pool(name="wop", bufs=DT))
        apool = ep(tc.tile_pool(name="apool", bufs=2))
        atmpp = ep(tc.tile_pool(name="atmpp", bufs=8))
        astrp = ep(tc.tile_pool(name="astrp", bufs=DT))

        attn_stack = ExitStack()
        ap2 = attn_stack.enter_context
        mpool = ap2(tc.tile_pool(name="memp", bufs=DT))
        qkp = ap2(tc.tile_pool(name="qk", bufs=8))
        vp = ap2(tc.tile_pool(name="vp", bufs=2 * ST))
        wqkp = ap2(tc.tile_pool(name="wqk", bufs=6 * DT))
        wvp = ap2(tc.tile_pool(name="wvp", bufs=2 * DT))

        # ---- P0/P1: loads + self QKV ----
        x_fm = []
        with tc.tile_pool(name="xp", bufs=DT) as xpool:
            x_fm = [xpool.tile([128, S], BF16, name="xfm", tag="x", bufs=DT)
                    for _ in range(DT)]

            def load_x_chunk(sc):
                for d in range(DT):
                    LOAD_ENG[d % len(LOAD_ENG)].dma_start(
                        out=x_fm[d][:, sc * CW:(sc + 1) * CW],
                        in_=xT[d * 128:(d + 1) * 128, sc * CW:(sc + 1) * CW])

            # x chunk 0 first, then the self-QKV weights (so the first
            # projection matmul is not gated behind the full 4MB of x),
            # then the remaining chunks
            load_x_chunk(0)

            z16 = [z16p.tile([128, SW], BF16, name="z16", tag="z16", bufs=DT)
                   for _ in range(DT)]

            wq_t = load_w_small(wqkp, wq, DC, "wqt")
            wk_t = load_w_small(wqkp, wk, DC, "wkt")
            wv_t = load_w_small(wvp, wv, DCA, "wvt")
            for sc in range(1, NCH):
                load_x_chunk(sc)
            wkx_t = load_w_small(wqkp, wkx, DC, "wkxt")
            wvx_t = load_w_small(wvp, wvx, DCA, "wvxt")

            q_s = [qkp.tile([128, S], BF16, name="qs", tag="qk", bufs=8)
                   for _ in range(2)]
            k_s = [qkp.tile([128, S], BF16, name="ks", tag="qk", bufs=8)
                   for _ in range(2)]
            with tc.tile_pool(name="ps1", bufs=2, space="PSUM") as ps1, \
                 tc.tile_pool(name="ps1v", bufs=2, space="PSUM") as ps1v:
                project_qk(q_s, wq_t, bq_t, x_fm, ps1, "qs")
                project_qk(k_s, wk_t, bk_t, x_fm, ps1, "ks")
                v_s = project_v(vp, wv_t, bva_t, x_fm, ps1v, S, "vs")

            z32 = []
            for d in range(DT):
                t = zp.tile([128, SW], F32R, name="z32", tag="z32", bufs=DT)
                LOAD_ENG[d % len(LOAD_ENG)].dma_start(
                    out=t[:], in_=xs[d * 128:(d + 1) * 128, :])
                z32.append(t)
            m_fm = []
            for d in range(DT):
                t = mpool.tile([128, M], BF16, name="mfm", tag="m", bufs=DT)
                LOAD_ENG[d % len(LOAD_ENG)].dma_start(
                    out=t[:], in_=memT[d * 128:(d + 1) * 128, :])
                m_fm.append(t)

        # x_fm freed here (pool closed); z32 holds the residual strip.

        # ---- P2/P3: self attention; cross K/V fills collective windows ----
        kx = [qkp.tile([128, M], BF16, name="kx", tag="qk", bufs=8)
              for _ in range(2)]
        vx = []

        def ckv_steps(psx, psxv, lo, hi):
            """Emit cross-K/V projection steps [lo, hi). Steps 0..3 are kx
            512-chunks; steps 4..11 are vx token-tile pairs."""
            for step in range(lo, hi):
                if step < 4:
                    sc = step
                    sl = slice(sc * CW, (sc + 1) * CW)
                    for t in range(2):
                        ps = psx.tile([128, CW], F32, name="kxps", tag="kxps",
                                      bufs=2)
                        for d in range(DT):
                            nc.tensor.matmul(
                                out=ps[:],
                                lhsT=wkx_t[d][:, t * 128:(t + 1) * 128],
                                rhs=m_fm[d][:, sl],
                                start=(d == 0), stop=(d == DT - 1))
                        nc.vector.tensor_scalar_add(out=kx[t][:, sl],
                                                    in0=ps[:],
                                                    scalar1=bkx_t[t][:])
                else:
                    g2 = step - 4
                    for s_t in (2 * g2, 2 * g2 + 1):
                        ps = psxv.tile([128, DCA], F32, name="vxps", tag="vxps",
                                       bufs=2)
                        for d in range(DT):
                            nc.tensor.matmul(
                                out=ps[:],
                                lhsT=m_fm[d][:, s_t * 128:(s_t + 1) * 128],
                                rhs=wvx_t[d][:],
                                start=(d == 0), stop=(d == DT - 1))
                        vt = vp.tile([128, DCA], BF16, name="vx", tag="vs",
                                     bufs=2 * ST)
                        nc.vector.tensor_tensor(out=vt[:], in0=ps[:],
                                                in1=bvxa_t[:], op=ALU.add)
                        vx.append(vt)

        with tc.tile_pool(name="pp1", bufs=7) as pp1, \
             tc.tile_pool(name="dn1", bufs=6) as dn1, \
             tc.tile_pool(name="ps2s", bufs=4, space="PSUM") as ps2s, \
             tc.tile_pool(name="ps2v", bufs=4, space="PSUM") as ps2v:
            a_sa = attention(apool, pp1, dn1, q_s, k_s, v_s, S, causal,
                             ps2s, ps2v, "sa")

        # ---- P4/P5: A2A self (cross-KV runs during it), out-proj, LN1, AG ----
        a2a_send(a_sa, t1in, t1out)
        with tc.tile_pool(name="psx", bufs=2, space="PSUM") as psx, \
             tc.tile_pool(name="psxv", bufs=2, space="PSUM") as psxv:
            ckv_steps(psx, psxv, 0, 6)
        wo_t = load_w_rows(wop, wo, D, "wot", DT)
        with tc.tile_pool(name="ps3", bufs=2, space="PSUM") as ps3, \
             tc.tile_pool(name="sqA", bufs=2) as sqA, \
             tc.tile_pool(name="tmA", bufs=4) as tmA:
            a_str = a2a_recv(t1out, atmpp, astrp, "a1")
            out_project_strip(wo_t, bo_t, a_str, z32, ps3, "o1")
            layer_norm_strip(z32, z16, 0, ps3, sqA, tmA, "l1")
        # cross-Q: project full q on my strip, then AllToAll to head-shard it
        with tc.tile_pool(name="wqxp", bufs=DT) as wqxp, \
             tc.tile_pool(name="qfp", bufs=DT) as qfp, \
             tc.tile_pool(name="ps5q", bufs=2, space="PSUM") as ps5q:
            wqxf_t = load_w_rows(wqxp, wqx, D, "wqxf", DT)
            qf = []
            for pt in range(DT):
                ps = ps5q.tile([128, CW], F32, name="qfps", tag="qf", bufs=2)
                for d in range(DT):
                    nc.tensor.matmul(
                        out=ps[:], lhsT=wqxf_t[d][:, pt * 128:(pt + 1) * 128],
                        rhs=z16[d][:],
                        start=(d == 0), stop=(d == DT - 1))
                t = qfp.tile([128, SW], BF16, name="qf", tag="qf", bufs=DT)
                nc.scalar.activation(out=t[:], in_=ps[:], func=AF.Identity,
                                     bias=bqx_t[pt][:], scale=1.0)
                qf.append(t)
            for j in range(2 * G):
                for u in range(2):
                    DMA_ENG[(2 * j + u) % len(DMA_ENG)].dma_start(
                        out=q2in[j * 256 + u * 128: j * 256 + (u + 1) * 128, :],
                        in_=qf[2 * (j % G) + u][:])
            nc.gpsimd.collective_compute(
                "AllToAll", ALU.bypass, replica_groups=GROUP8,
                ins=[q2in.opt()], outs=[q2out.opt()])
            # remaining cross-V work fills the AllToAll window
            with tc.tile_pool(name="psxB", bufs=2, space="PSUM") as psxB, \
                 tc.tile_pool(name="psxvB", bufs=2, space="PSUM") as psxvB:
                ckv_steps(psxB, psxvB, 6, 12)

        # ---- P6: assemble head-sharded cross Q from the A2A ----
        q_x = [qkp.tile([128, S], BF16, name="qx", tag="qk", bufs=8)
               for _ in range(2)]
        for u in range(2):
            for c in range(G):
                top = atmpp.tile([128, SW], BF16, name="qxt", tag="atmp",
                                 bufs=8)
                bot = atmpp.tile([128, SW], BF16, name="qxb", tag="atmp",
                                 bufs=8)
                DMA_ENG[c % len(DMA_ENG)].dma_start(
                    out=top[:],
                    in_=q2out[256 * c + 128 * u: 256 * c + 128 * (u + 1), :])
                DMA_ENG[(c + 1) % len(DMA_ENG)].dma_start(
                    out=bot[:],
                    in_=q2out[D + 256 * c + 128 * u:
                              D + 256 * c + 128 * (u + 1), :])
                slc = slice(c * SW, (c + 1) * SW)
                nc.vector.tensor_scalar_mul(out=q_x[u][:, slc], in0=top[:],
                                            scalar1=gmask_t[:, 0:1])
                nc.vector.scalar_tensor_tensor(
                    out=q_x[u][:, slc], in0=bot[:], scalar=gmask_t[:, 1:2],
                    in1=q_x[u][:, slc], op0=ALU.mult, op1=ALU.add)
        with tc.tile_pool(name="pp2", bufs=7) as pp2, \
             tc.tile_pool(name="dn2", bufs=6) as dn2, \
             tc.tile_pool(name="ps6s", bufs=4, space="PSUM") as ps6s, \
             tc.tile_pool(name="ps6v", bufs=4, space="PSUM") as ps6v:
            a_cx = attention(apool, pp2, dn2, q_x, kx, vx, M, False,
                             ps6s, ps6v, "cx")

        attn_stack.close()  # frees mem, q/k/v, weights for qkv

        # ---- P7/P8: A2A cross, out-proj, LN2 ----
        a2a_send(a_cx, t3in, t3out)
        wox_t = load_w_rows(wop, wox, D, "wot", DT)
        with tc.tile_pool(name="ps7", bufs=2, space="PSUM") as ps7, \
             tc.tile_pool(name="sqB", bufs=2) as sqB, \
             tc.tile_pool(name="tmB", bufs=4) as tmB:
            ax_str = a2a_recv(t3out, atmpp, astrp, "a3")
            out_project_strip(wox_t, box_t, ax_str, z32, ps7, "o2")
            layer_norm_strip(z32, z16, 1, ps7, sqB, tmB, "l2")

        # ---- P9: FFN + LN3 + output ----
        with tc.tile_pool(name="hp", bufs=FT) as hp, \
             tc.tile_pool(name="w1sp", bufs=FT) as w1sp, \
             tc.tile_pool(name="w2p", bufs=6) as w2p, \
             tc.tile_pool(name="sqC", bufs=2) as sqC, \
             tc.tile_pool(name="tmC", bufs=4) as tmC:
            h_t = [None] * FT
            # FF1 in f-blocks of 8: stream w1 row-tiles [128(d), 1024(f-blk)]
            with tc.tile_pool(name="ps9a", bufs=8, space="PSUM") as ps9a:
                for fb in range(FT // 8):
                    f1ps = [ps9a.tile([128, CW], F32, name=f"f1ps{i}",
                                      tag=f"f1ps{i}", bufs=1)
                            for i in range(8)]
                    for d in range(DT):
                        w1t = w1sp.tile([128, 1024], BF16, name="w1t", tag="w1",
                                        bufs=FT)
                        DMA_ENG[d % len(DMA_ENG)].dma_start(
                            out=w1t[:],
                            in_=w1[d * 128:(d + 1) * 128,
                                   fb * 1024:(fb + 1) * 1024])
                        for i in range(8):
                            nc.tensor.matmul(
                                out=f1ps[i][:],
                                lhsT=w1t[:, i * 128:(i + 1) * 128],
                                rhs=z16[d][:],
                                start=(d == 0), stop=(d == DT - 1))
                    for i in range(8):
                        f = fb * 8 + i
                        ht = hp.tile([128, CW], BF16, name="ht", tag="h",
                                     bufs=FT)
                        nc.scalar.activation(out=ht[:], in_=f1ps[i][:],
                                             func=AF.Relu, bias=b1_t[f][:],
                                             scale=1.0)
                        h_t[f] = ht
            # FF2: f-outer accumulation into 8 concurrent psum banks
            with tc.tile_pool(name="ps9b", bufs=8, space="PSUM") as ps9b:
                f2ps = [ps9b.tile([128, CW], F32, name=f"f2ps{d}",
                                  tag=f"f2ps{d}", bufs=1) for d in range(DT)]
                for f in range(FT):
                    w2t = w2p.tile([128, D], BF16, name="w2t", tag="w2", bufs=6)
                    DMA_ENG[f % len(DMA_ENG)].dma_start(
                        out=w2t[:], in_=w2[f * 128:(f + 1) * 128, :])
                    for d in range(DT):
                        nc.tensor.matmul(
                            out=f2ps[d][:], lhsT=w2t[:, d * 128:(d + 1) * 128],
                            rhs=h_t[f][:],
                            start=(f == 0), stop=(f == FT - 1))
                for d in range(DT):
                    nc.vector.scalar_tensor_tensor(
                        out=z32[d][:], in0=f2ps[d][:], scalar=b2_t[d][:],
                        in1=z32[d][:], op0=ALU.add, op1=ALU.add)
            with tc.tile_pool(name="ps9c", bufs=2, space="PSUM") as ps9c:
                layer_norm_strip(z32, None, 2, ps9c, sqC, tmC, "l3")
            for d in range(DT):
                DMA_ENG[d % len(DMA_ENG)].dma_start(
                    out=outT[d * 128:(d + 1) * 128, :], in_=z32[d][:])

    nc.finalize()
    return nc


def _get_nc(S, M, causal):
    key = (S, M, causal)
    if key not in _nc_cache:
        _nc_cache[key] = _build(S, M, causal)
    return _nc_cache[key]


def _prep_inputs(c, S, M, tgt, memory, Wqkv, bqkv, Wo_sa, bo_sa, Wq, bq, Wk, bk,
                 Wv, bv, Wo_cx, bo_cx, W1, b1, W2, b2, g_mha, bn_mha, g_crx,
                 bn_crx, g_ffn, bn_ffn):
    r, b = c % G, c // G
    hsl = slice(r * DC, (r + 1) * DC)
    f32 = np.float32
    bf16 = ml_dtypes.bfloat16

    def aug_v(wv_c, bv_c):
        wva = np.zeros((D, DCA), f32)
        bva = np.zeros((1, DCA), f32)
        for h in range(HL):
            wva[:, h * 65:h * 65 + 64] = wv_c[:, h * 64:(h + 1) * 64]
            bva[0, h * 65:h * 65 + 64] = bv_c[h * 64:(h + 1) * 64]
            bva[0, h * 65 + 64] = 1.0
        return wva, bva

    scale = np.float32(1.0 / np.sqrt(HD))
    wqkv_h = Wqkv.reshape(D, H, 3 * HD)
    bqkv_h = bqkv.reshape(H, 3 * HD)
    gh = slice(r * HL, (r + 1) * HL)
    wq_sa = wqkv_h[:, gh, 0:HD].reshape(D, DC) * scale
    wk_sa = wqkv_h[:, gh, HD:2 * HD].reshape(D, DC)
    wv_sa = wqkv_h[:, gh, 2 * HD:3 * HD].reshape(D, DC)
    bq_sa = bqkv_h[gh, 0:HD].reshape(DC) * scale
    bk_sa = bqkv_h[gh, HD:2 * HD].reshape(DC)
    bv_sa = bqkv_h[gh, 2 * HD:3 * HD].reshape(DC)
    wva_sa, bva_sa = aug_v(wv_sa, bv_sa)
    wvx_c, bvx_c = aug_v(Wv[:, hsl], bv[hsl])
    xT_full = np.ascontiguousarray(tgt[b].T)
    g0 = np.float32(1.0 if b == 0 else 0.0)
    gmask = np.broadcast_to(np.array([g0, 1.0 - g0], f32), (128, 2))
    m4 = np.zeros(4, f32)
    m4[(0 if r < 2 else 2) + b] = 1.0
    gmask4 = np.broadcast_to(m4, (128, 4))
    return {
        "xT": xT_full.astype(bf16),
        "xs": np.ascontiguousarray(xT_full[:, r * SW:(r + 1) * SW], f32),
        "memT": np.ascontiguousarray(memory[b].T).astype(bf16),
        "wq": wq_sa.astype(bf16),
        "wk": np.ascontiguousarray(wk_sa).astype(bf16),
        "wv": wva_sa.astype(bf16),
        "bq": np.ascontiguousarray(bq_sa.reshape(DC, 1), f32),
        "bk": np.ascontiguousarray(bk_sa.reshape(DC, 1), f32),
        "bva": bva_sa,
        "wo": Wo_sa.astype(bf16),
        "bo": np.ascontiguousarray(bo_sa.reshape(D, 1), f32),
        "wqx": (Wq * scale).astype(bf16),
        "wkx": np.ascontiguousarray(Wk[:, hsl]).astype(bf16),
        "wvx": wvx_c.astype(bf16),
        "bqx": np.ascontiguousarray((bq * scale).reshape(D, 1), f32),
        "bkx": np.ascontiguousarray(bk[hsl].reshape(DC, 1), f32),
        "bvxa": bvx_c,
        "wox": Wo_cx.astype(bf16),
        "box": np.ascontiguousarray(bo_cx.reshape(D, 1), f32),
        "w1": W1.astype(bf16),
        "b1": np.ascontiguousarray(b1.reshape(FF, 1), f32),
        "w2": W2.astype(bf16),
        "b2": np.ascontiguousarray(b2.reshape(D, 1), f32),
        "lng": np.ascontiguousarray(
            np.concatenate([g_mha, g_crx, g_ffn]).reshape(3 * D, 1), f32),
        "lnb": np.ascontiguousarray(
            np.concatenate([bn_mha, bn_crx, bn_ffn]).reshape(3 * D, 1), f32),
        "ones": np.full((128, 128), 1.0 / D, f32),
        "gmask": np.ascontiguousarray(gmask, f32),
        "gmask4": np.ascontiguousarray(gmask4, f32),
    }


def kernel(**inputs):
    tgt = np.asarray(inputs["tgt"], np.float32)
    memory = np.asarray(inputs["memory"], np.float32)
    mask = np.asarray(inputs["tgt_mask"])
    S, M = tgt.shape[1], memory.shape[1]

    if mask.any():
        expect = np.triu(np.ones((S, S), bool), 1)
        if not np.array_equal(mask, expect):
            raise NotImplementedError("only causal or empty tgt_mask supported")
        causal = True
    else:
        causal = False

    nc = _get_nc(S, M, causal)
    args = {k: np.asarray(v, np.float32) for k, v in inputs.items()
            if k not in ("tgt", "memory", "tgt_mask")}
    in_maps = [_prep_inputs(c, S, M, tgt, memory, **args) for c in range(NCORES)]

    trace = bool(int(os.environ.get("BASS_KERNEL_TRACE", "0")))
    res = run_bass_kernel_spmd(nc, in_maps, list(range(NCORES)), trace=trace)
    if trace:
        kernel.last_exec_time_ns = res.exec_time_ns
        kernel.last_result = res
    out = np.empty((B, S, D), np.float32)
    for c in range(NCORES):
        r, b = c % G, c // G
        out[b, r * SW:(r + 1) * SW, :] = res.results[c]["outT"].T
    return out
